# revision 70
# baseline (speedup 1.0000x reference)
"""AdaptiveGaussianConvLayer Trainium2 kernel (8 NeuronCores, SPMD, no collectives).

Math: out[b, j, d] = sum_i V[b, i, d] * W[b, i, j],
      W[b, i, j] = exp(-0.5 * ((j - i - mu[b,i]) / sigma[b,i])^2)
with B=4, N=4096, D=512; sigma in (0.5, 2.5), mu ~ 3*N(0,1).

W underflows to exactly 0.0 in fp32 once |j - i - mu|/sigma >= ~13.2, i.e. for
|j - i| >= ~48.  On a 64-shifted slab grid (slab s = rows [128s - 64, 128s +
64) of the core's j-range), each 128-wide j-tile t needs only slabs {t, t+1},
so the banded result matches the dense reference to fp32 rounding.

Sharding: 8 cores = (batch b) x (j-half h).  Core c computes
out[b, h*2048:(h+1)*2048, :].  Host pads V/sigma/mu with 64 zero rows on each
side of the core's i-window so all cores run one identical SPMD program.

Single-pass W on ACT: Derivative_Erf(x) = (2/sqrt(pi)) * exp(-x^2), so with
x = z/sqrt(2):  W = (sqrt(pi)/2) * Derivative_Erf(z / sqrt(2)).  ACT computes
f(scale*u + bias) with per-partition scale/bias, so one activation per slab
(scale r' = 1/(sigma*sqrt(2)), bias b0' = (-64 - p - mu) * r') produces the
slab's W directly in bf16 — no Square pass, no Exp pass, no z2 buffers.  The
sqrt(pi)/2 correction is folded into V on the host (V is pre-cast to bf16
there anyway, halving its DMA traffic).

Output is written in bf16 (the matmul already runs in bf16; measured rel err
~5e-4 vs the 2e-2 gate), halving out-DMA bytes; the host upcasts to fp32.

Per-core dataflow (i on partitions, j/d on the free axis):
  W slab s = DErf(r'_s * iota + b0'_s)        (ACT, bf16 out, 17 instrs,
             over the live 224-col window; edge strips pre-zeroed by two
             strided gpsimd memsets; iota generated on-chip)
  psum t   = sum_{k=0,1} W[slab t+k].T @ V[slab t+k]   (TensorE, K=128 bf16)
  obuf     <- psum bf16 copy (DVE tiles 0-12; ACT tiles 13-15 after its
              last W slab), out-DMA'd in pairs alternating the sync/gpsimd
              queues (behind V, so V keeps priority), last 4 tiles solo
The b0'/r' params ride as raw f32 bytes inside the head V DMA (bitcast f32
view on-chip; an explicit 1-col tracked ACT copy gates the W chain on the
head semaphore, since Tile does not track bitcast reads).  A few scratch
matmuls warm the PE clock gate before the real stream begins.
"""

import os
import numpy as np
import ml_dtypes

import concourse.bass as bass
import concourse.bacc as bacc
import concourse.mybir as mybir
import concourse.tile as tile
from concourse.bass_utils import run_bass_kernel_spmd

AF = mybir.ActivationFunctionType
ALU = mybir.AluOpType

B, N, D = 4, 4096, 512
NCORES = 8
HALF = N // 2             # 2048 j per core
NSLAB = HALF // 128 + 1   # 17 slabs of 128 rows on the 64-shifted grid
VROWS = NSLAB * 128       # 2176
JT = HALF // 128          # 16 j-tiles per core
WWIN = 256                # j-window width per slab
CW = 2 * NSLAB            # b0'/r' param columns (f32), shipped inside Vp
PADC = 2 * CW             # ... as bf16-encoded raw bytes at Vp's front

SQRT2 = float(np.sqrt(2.0))
WSCALE = float(np.sqrt(np.pi) / 2.0)

# genuinely used j-window per slab (edge slabs serve one j-tile)
def _slab_win(s):
    t_lo, t_hi = max(s - 1, 0), min(s, JT - 1)
    lo = (t_lo - (s - 1)) * 128
    return lo, (t_hi - t_lo + 1) * 128

WARMUP = int(os.environ.get("AGC_WARMUP", "30"))
FLATBAR = os.environ.get("AGC_FLATBAR", "1") == "1"

_cached = {}


def _flat_start_barrier(self, *, sem_only=False):
    """Flat all-engine barrier: every engine incs one sem and waits for the
    full count — one cross-engine hop instead of the stock sequential chain."""
    arrive = self.alloc_semaphore("flat_barrier_arrive")
    n = len(self.engines)
    for eng in self.engines.values():
        eng.sem_inc(arrive, 1)
    for eng in self.engines.values():
        eng.wait_ge(arrive, n)
    if not hasattr(self, "_flat_barrier_sems"):
        self._flat_barrier_sems = []
    self._flat_barrier_sems.append(arrive)


_stock_drain_and_barrier = tile.TileContext._drain_and_barrier


def _tail_drain_and_barrier(self, tick_clock, wait_clock):
    """Stock tail (its barrier instructions order the in-flight DMA completion
    sems ahead of the clears) + clear the flat-start-barrier sem so
    re-execution starts from zero."""
    _stock_drain_and_barrier(self, tick_clock, wait_clock)
    nc = self.nc
    fs = getattr(nc, "_flat_barrier_sems", [])
    if fs:
        nc.clear_and_free_semaphores(fs)
        nc._flat_barrier_sems = []


_stock_iatl = bacc.Bacc.insert_act_table_loads


def _single_table_iatl(self):
    """The stock pass emits an unconditional set-0 ACT table load at block
    entry (1.28us on ACT's critical path) ahead of the erf_derivative load
    the kernel actually needs.  Every activation here (Derivative_Erf, Copy)
    lives in the erf_derivative set, so the set-0 load is dead — drop it."""
    _stock_iatl(self)
    for b in self.main_func.blocks:
        keep = [i for i in b.instructions
                if not (isinstance(i, mybir.InstLoadActFuncSet)
                        and i.act_func_set_id == 0)]
        if len(keep) != len(b.instructions):
            b.instructions[:] = keep


def build_nc():
    tile.TileContext._drain_and_barrier = _tail_drain_and_barrier
    f32 = mybir.dt.float32
    bf16 = mybir.dt.bfloat16
    orig_barrier = bass.Bass.all_engine_barrier
    if FLATBAR:
        bass.Bass.all_engine_barrier = _flat_start_barrier
    try:
        nc = bacc.Bacc("TRN2", target_bir_lowering=False, debug=False)
    finally:
        bass.Bass.all_engine_barrier = orig_barrier

    # V pre-scaled by sqrt(pi)/2, pre-cast to bf16 AND pre-tiled partition-
    # major on the host: Vp[p, PADC + s*D+d] = V[row 128s+p, d] — every
    # partition is one contiguous run per DMA slice.  The first PADC bf16
    # columns are the f32 (b0', r') activation params as raw bytes, so the
    # single head DMA (params + V slabs 0-1) unblocks both W0 and MM0 with
    # one semaphore — per-queue wake-up latency varies 0.3-3us run to run,
    # so the head must not chain two DMAs.
    vp_d = nc.dram_tensor("Vp", [128, PADC + NSLAB * D], bf16, kind="ExternalInput").ap()
    # out is partition-major like Vp: out[p, t*D+d] = out_row(128t+p, d).
    # Per-partition contiguous runs double the out-DMA descriptor size
    # (2KB pairs); the host un-permutes in gather().
    out_d = nc.dram_tensor("out", [128, JT * D], bf16, kind="ExternalOutput").ap()

    with tile.TileContext(nc) as tc:
        with (
            tc.tile_pool(name="const", bufs=1) as constp,
            tc.tile_pool(name="big", bufs=1) as bigp,
            tc.tile_pool(name="ps", bufs=8, space=bass.MemorySpace.PSUM) as pspool,
            tc.tile_pool(name="obuf", bufs=5) as opool,
        ):
            vball = bigp.tile([128, PADC + NSLAB * D], bf16, name="vball")
            vbuf = vball[:, PADC : PADC + NSLAB * D]
            cst_hv = vball[:, 0:PADC].bitcast(f32)

            b0r = lambda s: (cst_hv[:, 2 * s : 2 * s + 1],
                             cst_hv[:, 2 * s + 1 : 2 * s + 2])

            wbuf = bigp.tile([128, NSLAB * WWIN], bf16, name="wbuf")

            # Descriptor generation (SWDGE) costs ~1us per DMA instruction,
            # serialized per queue.  Head DMA (params + V slabs 0-1) on the
            # sync ring; the rest of V is spread over the scalar + gpsimd
            # queues so three SWDGE chains run concurrently.
            nc.sync.dma_start(vball[:, 0 : PADC + D],
                              vp_d[:, 0 : PADC + D])
            nc.sync.dma_start(vball[:, PADC + D : PADC + 3 * D],
                              vp_d[:, PADC + D : PADC + 3 * D])
            nc.scalar.dma_start(vball[:, PADC + 3 * D : PADC + 7 * D],
                                vp_d[:, PADC + 3 * D : PADC + 7 * D])

            # force the erf_derivative ACT table load now (it is inserted
            # right before the first activation in ACT program order; with a
            # no-dependency dummy here it runs while cst is in flight
            # instead of after the cst semaphore wait)
            dummy = constp.tile([128, 1], f32, name="dummy")
            nc.scalar.activation(dummy[:], dummy[:], AF.Derivative_Erf)

            # PE warm-up operands, memset first on gpsimd (ready ~1us before
            # DVE dispatches its first instruction) so the warm-up stream
            # starts at ~6.7us and holds the PE clock ramp without a gap
            wscr = bigp.tile([128, 128], bf16, name="wscr")
            nc.gpsimd.memset(wscr[:], 0.0)
            wscr2 = bigp.tile([128, 128], bf16, name="wscr2")
            nc.gpsimd.memset(wscr2[:], 0.0)

            # iota source row for the W activations, generated on-chip
            # (fp32 is exact for 0..255); needed by W0 at ~9.5us, ready ~8.5
            iota_t = constp.tile([128, WWIN], f32, name="iota_t")
            nc.gpsimd.iota(iota_t[:], [[1, WWIN]], base=0, channel_multiplier=0,
                           allow_small_or_imprecise_dtypes=True)

            # W[p, c] is identically zero for window cols [0,16) and
            # [240,256) of every slab (|c - 64 - p - mu| <= 13.2*sigma is
            # unreachable there), so the per-slab activation covers only the
            # live 224 columns and two strided memsets zero the edge strips
            # once up front (gpsimd, done well before the first matmul)
            nc.gpsimd.memset(wbuf[:].rearrange("p (s c) -> p s c", c=WWIN)[:, :, 0:16], 0.0)
            nc.gpsimd.memset(wbuf[:].rearrange("p (s c) -> p s c", c=WWIN)[:, :, WWIN - 16 : WWIN], 0.0)

            # V tail on gpsimd FIRST (its SWDGE chain starts ~1.5us earlier
            # than if iota/strips ran first), slab order = consumption
            # order; out pairs ride behind V on the sync/gpsimd queues, so
            # V keeps priority
            for eng, lo, hi in ((nc.gpsimd, 7, 12), (nc.gpsimd, 12, 17)):
                eng.dma_start(vbuf[:, lo * D : hi * D],
                              vp_d[:, PADC + lo * D : PADC + hi * D])

            # PE warm-up: many SMALL (128-wide) scratch matmuls on zeros.
            # Fine granularity means the real matmul stream starts at most
            # ~250ns after its inputs are ready (vs 630ns with 512-wide
            # warmups) while PE stays continuously busy through the ~3.5us
            # clock-gate ramp, so real matmuls run at full rate immediately.
            wps = pspool.tile([128, D], f32, tag="ps", name="wps")
            for _ in range(WARMUP):
                nc.tensor.matmul(wps[:, 0:128], wscr[:], wscr2[:],
                                 start=True, stop=True)

            # W slab s in one ACT pass: DErf(r'*u + b0') = (2/sqrt(pi)) *
            # exp(-((u - 64 - p - mu)/sigma)^2 / 2)
            def emit_w(s):
                lo, w = _slab_win(s)
                a, b = max(lo, 16), min(lo + w, WWIN - 16)
                b0, r = b0r(s)
                nc.scalar.activation(
                    wbuf[:, s * WWIN + a : s * WWIN + b],
                    iota_t[:, a:b],
                    AF.Derivative_Erf, bias=b0, scale=r)



            def emit_jtile(t, ps):
                out = ps[:]
                for k in range(2):
                    ls = t + k
                    nc.tensor.matmul(
                        out,
                        wbuf[:, ls * WWIN + (1 - k) * 128 : ls * WWIN + (2 - k) * 128],
                        vbuf[:, ls * D : (ls + 1) * D],
                        start=(k == 0),
                        stop=(k == 1),
                    )

            # Tile does NOT track reads through bitcast APs — without an
            # explicit gate the W chain would race the head DMA and read
            # stale param bytes.  A tracked 1-column ACT copy of the head
            # region blocks ACT until the head semaphore fires.
            gate = constp.tile([128, 1], bf16, name="gate")
            nc.scalar.copy(gate[:], vball[:, 0:1])

            # pipeline: per-slab W -> j-tiles as they unlock -> PSUM->SBUF
            # bf16 copies (DVE tiles 0-12, ACT tiles 13-15 deferred until
            # after the last W slab so the W stream never stalls).
            # Out-DMA granularity: 3 quads + 1 pair + 2 singles = 6 DMA
            # instructions (descriptor generation is ~1us each, serialized
            # per queue — the old 10-instruction scheme spent ~4us of the
            # tail in SWDGE chains).  Quads are 4KB-contiguous per partition
            # in the partition-major out layout.
            emit_w(0)
            ps = ob = None
            deferred = []
            for s in range(1, NSLAB):
                emit_w(s)
                t = s - 1
                psp = pspool.tile([128, D], f32, tag="ps", name="ps")
                if t < 12:
                    if t % 4 == 0:
                        ob = opool.tile([128, 4 * D], bf16, name="ob")
                elif t in (12, 14):
                    ob = opool.tile([128, 2 * D], bf16, name="ob")
                emit_jtile(t, psp)
                if t >= 13:
                    deferred.append((t, psp, ob))
                else:
                    dst = (ob[:, (t % 4) * D : (t % 4 + 1) * D] if t < 12
                           else ob[:, 0:D])
                    nc.vector.tensor_copy(dst, psp[:])
                    if t == 3:
                        nc.sync.dma_start(out_d[:, 0 : 4 * D], ob[:])
                    elif t == 7:
                        nc.gpsimd.dma_start(out_d[:, 4 * D : 8 * D], ob[:])
                    elif t == 11:
                        nc.sync.dma_start(out_d[:, 8 * D : 12 * D], ob[:])
            for t, psp, ob in deferred:
                nc.scalar.activation(
                    ob[:, (t % 2) * D : (t % 2 + 1) * D], psp[:], AF.Copy)
                if t == 13:
                    nc.gpsimd.dma_start(out_d[:, 12 * D : 14 * D], ob[:])
                elif t == 14:
                    nc.sync.dma_start(out_d[:, 14 * D : 15 * D], ob[:, 0:D])
                else:
                    nc.gpsimd.dma_start(out_d[:, 15 * D : 16 * D],
                                        ob[:, D : 2 * D])

    bacc.Bacc.insert_act_table_loads = _single_table_iatl
    try:
        nc.compile()
    finally:
        bacc.Bacc.insert_act_table_loads = _stock_iatl
    return nc


def _get_nc():
    if "nc" not in _cached:
        _cached["nc"] = build_nc()
    return _cached["nc"]


def make_in_maps(V, sigma, mu):
    """Host-side sharding: per-core padded bf16 V rows + scale table."""
    V = np.asarray(V, dtype=np.float32)
    sigma = np.asarray(sigma, dtype=np.float32).reshape(B, N)
    mu = np.asarray(mu, dtype=np.float32).reshape(B, N)
    pidx = (np.arange(VROWS) % 128).astype(np.float32)
    in_maps = []
    for c in range(NCORES):
        b, h = divmod(c, 2)
        jb = h * HALF
        lo, hi = jb - 64, jb + HALF + 64
        slo, shi = max(lo, 0), min(hi, N)
        vp = np.zeros((VROWS, D), ml_dtypes.bfloat16)
        sig = np.ones(VROWS, np.float32)
        muv = np.zeros(VROWS, np.float32)
        vp[slo - lo : shi - lo] = (V[b, slo:shi] * WSCALE).astype(ml_dtypes.bfloat16)
        sig[slo - lo : shi - lo] = sigma[b, slo:shi]
        muv[slo - lo : shi - lo] = mu[b, slo:shi]
        r = (np.float32(1.0) / (sig * np.float32(SQRT2))).astype(np.float32)
        b0 = ((np.float32(-64.0) - pidx - muv) * r).astype(np.float32)
        cst = np.zeros((128, CW), np.float32)
        cst[:, 0 : 2 * NSLAB : 2] = b0.reshape(NSLAB, 128).T
        cst[:, 1 : 2 * NSLAB : 2] = r.reshape(NSLAB, 128).T
        vp2 = np.empty((128, PADC + NSLAB * D), ml_dtypes.bfloat16)
        # f32 params shipped as raw bytes in the bf16 tensor's first columns
        vp2[:, 0:PADC] = np.ascontiguousarray(cst).view(np.uint16).view(
            ml_dtypes.bfloat16)
        vp2[:, PADC:] = (
            vp.reshape(NSLAB, 128, D).transpose(1, 0, 2).reshape(128, NSLAB * D))
        in_maps.append({"Vp": vp2})
    return in_maps


def gather(results):
    out = np.empty((B, N, D), np.float32)
    for c in range(NCORES):
        b, h = divmod(c, 2)
        arr = np.asarray(results[c]["out"]).astype(np.float32)
        out[b, h * HALF : (h + 1) * HALF] = (
            arr.reshape(128, JT, D).transpose(1, 0, 2).reshape(HALF, D))
    return out


def kernel(V, sigma, mu):
    nc = _get_nc()
    in_maps = make_in_maps(V, sigma, mu)
    res = run_bass_kernel_spmd(nc, in_maps, core_ids=list(range(NCORES)))
    return gather(res.results)


# revision 71
# speedup vs baseline: 1.0022x; 1.0022x over previous
"""AdaptiveGaussianConvLayer Trainium2 kernel (8 NeuronCores, SPMD, no collectives).

Math: out[b, j, d] = sum_i V[b, i, d] * W[b, i, j],
      W[b, i, j] = exp(-0.5 * ((j - i - mu[b,i]) / sigma[b,i])^2)
with B=4, N=4096, D=512; sigma in (0.5, 2.5), mu ~ 3*N(0,1).

W underflows to exactly 0.0 in fp32 once |j - i - mu|/sigma >= ~13.2, i.e. for
|j - i| >= ~48.  On a 64-shifted slab grid (slab s = rows [128s - 64, 128s +
64) of the core's j-range), each 128-wide j-tile t needs only slabs {t, t+1},
so the banded result matches the dense reference to fp32 rounding.

Sharding: 8 cores = (batch b) x (j-half h).  Core c computes
out[b, h*2048:(h+1)*2048, :].  Host pads V/sigma/mu with 64 zero rows on each
side of the core's i-window so all cores run one identical SPMD program.

Single-pass W on ACT: Derivative_Erf(x) = (2/sqrt(pi)) * exp(-x^2), so with
x = z/sqrt(2):  W = (sqrt(pi)/2) * Derivative_Erf(z / sqrt(2)).  ACT computes
f(scale*u + bias) with per-partition scale/bias, so one activation per slab
(scale r' = 1/(sigma*sqrt(2)), bias b0' = (-64 - p - mu) * r') produces the
slab's W directly in bf16 — no Square pass, no Exp pass, no z2 buffers.  The
sqrt(pi)/2 correction is folded into V on the host (V is pre-cast to bf16
there anyway, halving its DMA traffic).

Output is written in bf16 (the matmul already runs in bf16; measured rel err
~5e-4 vs the 2e-2 gate), halving out-DMA bytes; the host upcasts to fp32.

Per-core dataflow (i on partitions, j/d on the free axis):
  W slab s = DErf(r'_s * iota + b0'_s)        (ACT, bf16 out, 17 instrs,
             over the live 224-col window; edge strips pre-zeroed by two
             strided gpsimd memsets; iota generated on-chip)
  psum t   = sum_{k=0,1} W[slab t+k].T @ V[slab t+k]   (TensorE, K=128 bf16)
  obuf     <- psum bf16 copy (DVE tiles 0-12; ACT tiles 13-15 after its
              last W slab), out-DMA'd as 3 quads + 1 pair + 2 singles
              alternating the sync/gpsimd queues behind V (descriptor
              generation is ~1us per DMA instruction, so few fat DMAs)
The b0'/r' params ride as raw f32 bytes inside the head V DMA (bitcast f32
view on-chip; an explicit 1-col tracked ACT copy gates the W chain on the
head semaphore, since Tile does not track bitcast reads).  A few scratch
matmuls warm the PE clock gate before the real stream begins.
"""

import os
import numpy as np
import ml_dtypes

import concourse.bass as bass
import concourse.bacc as bacc
import concourse.mybir as mybir
import concourse.tile as tile
from concourse.bass_utils import run_bass_kernel_spmd

AF = mybir.ActivationFunctionType
ALU = mybir.AluOpType

B, N, D = 4, 4096, 512
NCORES = 8
HALF = N // 2             # 2048 j per core
NSLAB = HALF // 128 + 1   # 17 slabs of 128 rows on the 64-shifted grid
VROWS = NSLAB * 128       # 2176
JT = HALF // 128          # 16 j-tiles per core
WWIN = 256                # j-window width per slab
CW = 2 * NSLAB            # b0'/r' param columns (f32), shipped inside Vp
PADC = 2 * CW             # ... as bf16-encoded raw bytes at Vp's front

SQRT2 = float(np.sqrt(2.0))
WSCALE = float(np.sqrt(np.pi) / 2.0)

# genuinely used j-window per slab (edge slabs serve one j-tile)
def _slab_win(s):
    t_lo, t_hi = max(s - 1, 0), min(s, JT - 1)
    lo = (t_lo - (s - 1)) * 128
    return lo, (t_hi - t_lo + 1) * 128

WARMUP = int(os.environ.get("AGC_WARMUP", "30"))
FLATBAR = os.environ.get("AGC_FLATBAR", "1") == "1"

_cached = {}


def _flat_start_barrier(self, *, sem_only=False):
    """Flat all-engine barrier: every engine incs one sem and waits for the
    full count — one cross-engine hop instead of the stock sequential chain."""
    arrive = self.alloc_semaphore("flat_barrier_arrive")
    n = len(self.engines)
    for eng in self.engines.values():
        eng.sem_inc(arrive, 1)
    for eng in self.engines.values():
        eng.wait_ge(arrive, n)
    if not hasattr(self, "_flat_barrier_sems"):
        self._flat_barrier_sems = []
    self._flat_barrier_sems.append(arrive)


_stock_drain_and_barrier = tile.TileContext._drain_and_barrier


def _tail_drain_and_barrier(self, tick_clock, wait_clock):
    """Stock tail (its barrier instructions order the in-flight DMA completion
    sems ahead of the clears) + clear the flat-start-barrier sem so
    re-execution starts from zero."""
    _stock_drain_and_barrier(self, tick_clock, wait_clock)
    nc = self.nc
    fs = getattr(nc, "_flat_barrier_sems", [])
    if fs:
        nc.clear_and_free_semaphores(fs)
        nc._flat_barrier_sems = []


_stock_iatl = bacc.Bacc.insert_act_table_loads


def _single_table_iatl(self):
    """The stock pass emits an unconditional set-0 ACT table load at block
    entry (1.28us on ACT's critical path) ahead of the erf_derivative load
    the kernel actually needs.  Every activation here (Derivative_Erf, Copy)
    lives in the erf_derivative set, so the set-0 load is dead — drop it."""
    _stock_iatl(self)
    for b in self.main_func.blocks:
        keep = [i for i in b.instructions
                if not (isinstance(i, mybir.InstLoadActFuncSet)
                        and i.act_func_set_id == 0)]
        if len(keep) != len(b.instructions):
            b.instructions[:] = keep


def build_nc():
    tile.TileContext._drain_and_barrier = _tail_drain_and_barrier
    f32 = mybir.dt.float32
    bf16 = mybir.dt.bfloat16
    orig_barrier = bass.Bass.all_engine_barrier
    if FLATBAR:
        bass.Bass.all_engine_barrier = _flat_start_barrier
    try:
        nc = bacc.Bacc("TRN2", target_bir_lowering=False, debug=False)
    finally:
        bass.Bass.all_engine_barrier = orig_barrier

    # V pre-scaled by sqrt(pi)/2, pre-cast to bf16 AND pre-tiled partition-
    # major on the host: Vp[p, PADC + s*D+d] = V[row 128s+p, d] — every
    # partition is one contiguous run per DMA slice.  The first PADC bf16
    # columns are the f32 (b0', r') activation params as raw bytes, so the
    # single head DMA (params + V slabs 0-1) unblocks both W0 and MM0 with
    # one semaphore — per-queue wake-up latency varies 0.3-3us run to run,
    # so the head must not chain two DMAs.
    vp_d = nc.dram_tensor("Vp", [128, PADC + NSLAB * D], bf16, kind="ExternalInput").ap()
    # out is partition-major like Vp: out[p, t*D+d] = out_row(128t+p, d).
    # Per-partition contiguous runs double the out-DMA descriptor size
    # (2KB pairs); the host un-permutes in gather().
    out_d = nc.dram_tensor("out", [128, JT * D], bf16, kind="ExternalOutput").ap()

    with tile.TileContext(nc) as tc:
        with (
            tc.tile_pool(name="const", bufs=1) as constp,
            tc.tile_pool(name="big", bufs=1) as bigp,
            tc.tile_pool(name="ps", bufs=8, space=bass.MemorySpace.PSUM) as pspool,
            tc.tile_pool(name="obuf", bufs=5) as opool,
        ):
            vball = bigp.tile([128, PADC + NSLAB * D], bf16, name="vball")
            vbuf = vball[:, PADC : PADC + NSLAB * D]
            cst_hv = vball[:, 0:PADC].bitcast(f32)

            b0r = lambda s: (cst_hv[:, 2 * s : 2 * s + 1],
                             cst_hv[:, 2 * s + 1 : 2 * s + 2])

            wbuf = bigp.tile([128, NSLAB * WWIN], bf16, name="wbuf")

            # Descriptor generation (SWDGE) costs ~1us per DMA instruction,
            # serialized per queue.  Head DMA (params + V slabs 0-1) on the
            # sync ring; the rest of V is spread over the scalar + gpsimd
            # queues so three SWDGE chains run concurrently.
            nc.sync.dma_start(vball[:, 0 : PADC + D],
                              vp_d[:, 0 : PADC + D])
            nc.sync.dma_start(vball[:, PADC + D : PADC + 3 * D],
                              vp_d[:, PADC + D : PADC + 3 * D])
            nc.scalar.dma_start(vball[:, PADC + 3 * D : PADC + 7 * D],
                                vp_d[:, PADC + 3 * D : PADC + 7 * D])

            # force the erf_derivative ACT table load now (it is inserted
            # right before the first activation in ACT program order; with a
            # no-dependency dummy here it runs while cst is in flight
            # instead of after the cst semaphore wait)
            dummy = constp.tile([128, 1], f32, name="dummy")
            nc.scalar.activation(dummy[:], dummy[:], AF.Derivative_Erf)

            # PE warm-up operands, memset first on gpsimd (ready ~1us before
            # DVE dispatches its first instruction) so the warm-up stream
            # starts at ~6.7us and holds the PE clock ramp without a gap
            wscr = bigp.tile([128, 128], bf16, name="wscr")
            nc.gpsimd.memset(wscr[:], 0.0)
            wscr2 = bigp.tile([128, 128], bf16, name="wscr2")
            nc.gpsimd.memset(wscr2[:], 0.0)

            # iota source row for the W activations, generated on-chip
            # (fp32 is exact for 0..255); needed by W0 at ~9.5us, ready ~8.5
            iota_t = constp.tile([128, WWIN], f32, name="iota_t")
            nc.gpsimd.iota(iota_t[:], [[1, WWIN]], base=0, channel_multiplier=0,
                           allow_small_or_imprecise_dtypes=True)

            # W[p, c] is identically zero for window cols [0,16) and
            # [240,256) of every slab (|c - 64 - p - mu| <= 13.2*sigma is
            # unreachable there), so the per-slab activation covers only the
            # live 224 columns and two strided memsets zero the edge strips
            # once up front (gpsimd, done well before the first matmul)
            nc.gpsimd.memset(wbuf[:].rearrange("p (s c) -> p s c", c=WWIN)[:, :, 0:16], 0.0)
            nc.gpsimd.memset(wbuf[:].rearrange("p (s c) -> p s c", c=WWIN)[:, :, WWIN - 16 : WWIN], 0.0)

            # V tail on gpsimd FIRST (its SWDGE chain starts ~1.5us earlier
            # than if iota/strips ran first), slab order = consumption
            # order; out pairs ride behind V on the sync/gpsimd queues, so
            # V keeps priority
            for eng, lo, hi in ((nc.gpsimd, 7, 12), (nc.gpsimd, 12, 17)):
                eng.dma_start(vbuf[:, lo * D : hi * D],
                              vp_d[:, PADC + lo * D : PADC + hi * D])

            # PE warm-up: many SMALL (128-wide) scratch matmuls on zeros.
            # Fine granularity means the real matmul stream starts at most
            # ~250ns after its inputs are ready (vs 630ns with 512-wide
            # warmups) while PE stays continuously busy through the ~3.5us
            # clock-gate ramp, so real matmuls run at full rate immediately.
            wps = pspool.tile([128, D], f32, tag="ps", name="wps")
            for _ in range(WARMUP):
                nc.tensor.matmul(wps[:, 0:128], wscr[:], wscr2[:],
                                 start=True, stop=True)

            # W slab s in one ACT pass: DErf(r'*u + b0') = (2/sqrt(pi)) *
            # exp(-((u - 64 - p - mu)/sigma)^2 / 2)
            def emit_w(s):
                lo, w = _slab_win(s)
                a, b = max(lo, 16), min(lo + w, WWIN - 16)
                b0, r = b0r(s)
                nc.scalar.activation(
                    wbuf[:, s * WWIN + a : s * WWIN + b],
                    iota_t[:, a:b],
                    AF.Derivative_Erf, bias=b0, scale=r)



            def emit_jtile(t, ps):
                out = ps[:]
                for k in range(2):
                    ls = t + k
                    nc.tensor.matmul(
                        out,
                        wbuf[:, ls * WWIN + (1 - k) * 128 : ls * WWIN + (2 - k) * 128],
                        vbuf[:, ls * D : (ls + 1) * D],
                        start=(k == 0),
                        stop=(k == 1),
                    )

            # Tile does NOT track reads through bitcast APs — without an
            # explicit gate the W chain would race the head DMA and read
            # stale param bytes.  A tracked 1-column ACT copy of the head
            # region blocks ACT until the head semaphore fires.
            gate = constp.tile([128, 1], bf16, name="gate")
            nc.scalar.copy(gate[:], vball[:, 0:1])

            # pipeline: per-slab W -> j-tiles as they unlock -> PSUM->SBUF
            # bf16 copies (DVE tiles 0-12, ACT tiles 13-15 deferred until
            # after the last W slab so the W stream never stalls).
            # Out-DMA granularity: 3 quads + 1 pair + 2 singles = 6 DMA
            # instructions (descriptor generation is ~1us each, serialized
            # per queue — the old 10-instruction scheme spent ~4us of the
            # tail in SWDGE chains).  Quads are 4KB-contiguous per partition
            # in the partition-major out layout.
            emit_w(0)
            ps = ob = None
            deferred = []
            for s in range(1, NSLAB):
                emit_w(s)
                t = s - 1
                psp = pspool.tile([128, D], f32, tag="ps", name="ps")
                if t < 12:
                    if t % 4 == 0:
                        ob = opool.tile([128, 4 * D], bf16, name="ob")
                elif t in (12, 14):
                    ob = opool.tile([128, 2 * D], bf16, name="ob")
                emit_jtile(t, psp)
                if t >= 13:
                    deferred.append((t, psp, ob))
                else:
                    dst = (ob[:, (t % 4) * D : (t % 4 + 1) * D] if t < 12
                           else ob[:, 0:D])
                    nc.vector.tensor_copy(dst, psp[:])
                    if t == 3:
                        nc.sync.dma_start(out_d[:, 0 : 4 * D], ob[:])
                    elif t == 7:
                        nc.gpsimd.dma_start(out_d[:, 4 * D : 8 * D], ob[:])
                    elif t == 11:
                        nc.sync.dma_start(out_d[:, 8 * D : 12 * D], ob[:])
            for t, psp, ob in deferred:
                nc.scalar.activation(
                    ob[:, (t % 2) * D : (t % 2 + 1) * D], psp[:], AF.Copy)
                if t == 13:
                    nc.gpsimd.dma_start(out_d[:, 12 * D : 14 * D], ob[:])
                elif t == 14:
                    nc.sync.dma_start(out_d[:, 14 * D : 15 * D], ob[:, 0:D])
                else:
                    nc.gpsimd.dma_start(out_d[:, 15 * D : 16 * D],
                                        ob[:, D : 2 * D])

    bacc.Bacc.insert_act_table_loads = _single_table_iatl
    try:
        nc.compile()
    finally:
        bacc.Bacc.insert_act_table_loads = _stock_iatl
    return nc


def _get_nc():
    if "nc" not in _cached:
        _cached["nc"] = build_nc()
    return _cached["nc"]


def make_in_maps(V, sigma, mu):
    """Host-side sharding: per-core padded bf16 V rows + scale table."""
    V = np.asarray(V, dtype=np.float32)
    sigma = np.asarray(sigma, dtype=np.float32).reshape(B, N)
    mu = np.asarray(mu, dtype=np.float32).reshape(B, N)
    pidx = (np.arange(VROWS) % 128).astype(np.float32)
    in_maps = []
    for c in range(NCORES):
        b, h = divmod(c, 2)
        jb = h * HALF
        lo, hi = jb - 64, jb + HALF + 64
        slo, shi = max(lo, 0), min(hi, N)
        vp = np.zeros((VROWS, D), ml_dtypes.bfloat16)
        sig = np.ones(VROWS, np.float32)
        muv = np.zeros(VROWS, np.float32)
        vp[slo - lo : shi - lo] = (V[b, slo:shi] * WSCALE).astype(ml_dtypes.bfloat16)
        sig[slo - lo : shi - lo] = sigma[b, slo:shi]
        muv[slo - lo : shi - lo] = mu[b, slo:shi]
        r = (np.float32(1.0) / (sig * np.float32(SQRT2))).astype(np.float32)
        b0 = ((np.float32(-64.0) - pidx - muv) * r).astype(np.float32)
        cst = np.zeros((128, CW), np.float32)
        cst[:, 0 : 2 * NSLAB : 2] = b0.reshape(NSLAB, 128).T
        cst[:, 1 : 2 * NSLAB : 2] = r.reshape(NSLAB, 128).T
        vp2 = np.empty((128, PADC + NSLAB * D), ml_dtypes.bfloat16)
        # f32 params shipped as raw bytes in the bf16 tensor's first columns
        vp2[:, 0:PADC] = np.ascontiguousarray(cst).view(np.uint16).view(
            ml_dtypes.bfloat16)
        vp2[:, PADC:] = (
            vp.reshape(NSLAB, 128, D).transpose(1, 0, 2).reshape(128, NSLAB * D))
        in_maps.append({"Vp": vp2})
    return in_maps


def gather(results):
    out = np.empty((B, N, D), np.float32)
    for c in range(NCORES):
        b, h = divmod(c, 2)
        arr = np.asarray(results[c]["out"]).astype(np.float32)
        out[b, h * HALF : (h + 1) * HALF] = (
            arr.reshape(128, JT, D).transpose(1, 0, 2).reshape(HALF, D))
    return out


def kernel(V, sigma, mu):
    nc = _get_nc()
    in_maps = make_in_maps(V, sigma, mu)
    res = run_bass_kernel_spmd(nc, in_maps, core_ids=list(range(NCORES)))
    return gather(res.results)


# revision 72
# speedup vs baseline: 1.0686x; 1.0663x over previous
"""AdaptiveGaussianConvLayer Trainium2 kernel (8 NeuronCores, SPMD, no collectives).

Math: out[b, j, d] = sum_i V[b, i, d] * W[b, i, j],
      W[b, i, j] = exp(-0.5 * ((j - i - mu[b,i]) / sigma[b,i])^2)
with B=4, N=4096, D=512; sigma in (0.5, 2.5), mu ~ 3*N(0,1).

W underflows to exactly 0.0 in fp32 once |j - i - mu|/sigma >= ~13.2, i.e. for
|j - i| >= ~48.  On a 64-shifted slab grid (slab s = rows [128s - 64, 128s +
64) of the core's j-range), each 128-wide j-tile t needs only slabs {t, t+1},
so the banded result matches the dense reference to fp32 rounding.

Sharding: 8 cores = (batch b) x (j-half h).  Core c computes
out[b, h*2048:(h+1)*2048, :].  Host pads V/sigma/mu with 64 zero rows on each
side of the core's i-window so all cores run one identical SPMD program.

Single-pass W on ACT: Derivative_Erf(x) = (2/sqrt(pi)) * exp(-x^2), so with
x = z/sqrt(2):  W = (sqrt(pi)/2) * Derivative_Erf(z / sqrt(2)).  ACT computes
f(scale*u + bias) with per-partition scale/bias, so one activation per slab
(scale r' = 1/(sigma*sqrt(2)), bias b0' = (-64 - p - mu) * r') produces the
slab's W directly in bf16 — no Square pass, no Exp pass, no z2 buffers.  The
sqrt(pi)/2 correction is folded into V on the host (V is pre-cast to bf16
there anyway, halving its DMA traffic).

Output is written in bf16 (the matmul already runs in bf16; measured rel err
~5e-4 vs the 2e-2 gate), halving out-DMA bytes; the host upcasts to fp32.

Per-core dataflow (i on partitions, j/d on the free axis):
  W slab s = DErf(r'_s * iota + b0'_s)        (ACT, bf16 out, 17 instrs,
             over the live 224-col window; edge strips pre-zeroed by two
             strided gpsimd memsets; iota generated on-chip)
  psum t   = sum_{k=0,1} W[slab t+k].T @ V[slab t+k]   (TensorE, K=128 bf16)
  obuf     <- psum bf16 copy (DVE tiles 0-12; ACT tiles 13-15 after its
              last W slab), out-DMA'd as 3 quads + 1 pair + 2 singles
              alternating the sync/gpsimd queues behind V (descriptor
              generation is ~1us per DMA instruction, so few fat DMAs)
The b0'/r' params ride as raw f32 bytes inside the head V DMA (bitcast f32
view on-chip; an explicit 1-col tracked ACT copy gates the W chain on the
head semaphore, since Tile does not track bitcast reads).  A few scratch
matmuls warm the PE clock gate before the real stream begins.
"""

import os
import numpy as np
import ml_dtypes

import concourse.bass as bass
import concourse.bacc as bacc
import concourse.mybir as mybir
import concourse.tile as tile
from concourse.bass_utils import run_bass_kernel_spmd

AF = mybir.ActivationFunctionType
ALU = mybir.AluOpType

B, N, D = 4, 4096, 512
NCORES = 8
HALF = N // 2             # 2048 j per core
NSLAB = HALF // 128 + 1   # 17 slabs of 128 rows on the 64-shifted grid
VROWS = NSLAB * 128       # 2176
JT = HALF // 128          # 16 j-tiles per core
WWIN = 256                # j-window width per slab
CW = 2 * NSLAB            # b0'/r' param columns (f32), shipped inside Vp
PADC = 2 * CW             # ... as bf16-encoded raw bytes at Vp's front

SQRT2 = float(np.sqrt(2.0))
WSCALE = float(np.sqrt(np.pi) / 2.0)

# genuinely used j-window per slab (edge slabs serve one j-tile)
def _slab_win(s):
    t_lo, t_hi = max(s - 1, 0), min(s, JT - 1)
    lo = (t_lo - (s - 1)) * 128
    return lo, (t_hi - t_lo + 1) * 128

WARMUP = int(os.environ.get("AGC_WARMUP", "30"))
FLATBAR = os.environ.get("AGC_FLATBAR", "1") == "1"

_cached = {}


def _flat_start_barrier(self, *, sem_only=False):
    """Flat all-engine barrier: every engine incs one sem and waits for the
    full count — one cross-engine hop instead of the stock sequential chain."""
    arrive = self.alloc_semaphore("flat_barrier_arrive")
    n = len(self.engines)
    for eng in self.engines.values():
        eng.sem_inc(arrive, 1)
    for eng in self.engines.values():
        eng.wait_ge(arrive, n)
    if not hasattr(self, "_flat_barrier_sems"):
        self._flat_barrier_sems = []
    self._flat_barrier_sems.append(arrive)


_stock_drain_and_barrier = tile.TileContext._drain_and_barrier


def _tail_drain_and_barrier(self, tick_clock, wait_clock):
    """Stock tail (its barrier instructions order the in-flight DMA completion
    sems ahead of the clears) + clear the flat-start-barrier sem so
    re-execution starts from zero."""
    _stock_drain_and_barrier(self, tick_clock, wait_clock)
    nc = self.nc
    fs = getattr(nc, "_flat_barrier_sems", [])
    if fs:
        nc.clear_and_free_semaphores(fs)
        nc._flat_barrier_sems = []


_stock_iatl = bacc.Bacc.insert_act_table_loads


def _single_table_iatl(self):
    """The stock pass emits an unconditional set-0 ACT table load at block
    entry (1.28us on ACT's critical path) ahead of the erf_derivative load
    the kernel actually needs.  Every activation here (Derivative_Erf, Copy)
    lives in the erf_derivative set, so the set-0 load is dead — drop it."""
    _stock_iatl(self)
    for b in self.main_func.blocks:
        keep = [i for i in b.instructions
                if not (isinstance(i, mybir.InstLoadActFuncSet)
                        and i.act_func_set_id == 0)]
        if len(keep) != len(b.instructions):
            b.instructions[:] = keep


def build_nc():
    tile.TileContext._drain_and_barrier = _tail_drain_and_barrier
    f32 = mybir.dt.float32
    bf16 = mybir.dt.bfloat16
    orig_barrier = bass.Bass.all_engine_barrier
    if FLATBAR:
        bass.Bass.all_engine_barrier = _flat_start_barrier
    try:
        nc = bacc.Bacc("TRN2", target_bir_lowering=False, debug=False)
    finally:
        bass.Bass.all_engine_barrier = orig_barrier

    # V pre-scaled by sqrt(pi)/2, pre-cast to bf16 AND pre-tiled partition-
    # major on the host: Vp[p, PADC + s*D+d] = V[row 128s+p, d] — every
    # partition is one contiguous run per DMA slice.  The first PADC bf16
    # columns are the f32 (b0', r') activation params as raw bytes, so the
    # single head DMA (params + V slabs 0-1) unblocks both W0 and MM0 with
    # one semaphore — per-queue wake-up latency varies 0.3-3us run to run,
    # so the head must not chain two DMAs.
    vp_d = nc.dram_tensor("Vp", [128, PADC + NSLAB * D], bf16, kind="ExternalInput").ap()
    # out is partition-major like Vp: out[p, t*D+d] = out_row(128t+p, d).
    # Per-partition contiguous runs double the out-DMA descriptor size
    # (2KB pairs); the host un-permutes in gather().
    out_d = nc.dram_tensor("out", [128, JT * D], bf16, kind="ExternalOutput").ap()

    with tile.TileContext(nc) as tc:
        with (
            tc.tile_pool(name="const", bufs=1) as constp,
            tc.tile_pool(name="big", bufs=1) as bigp,
            tc.tile_pool(name="ps", bufs=8, space=bass.MemorySpace.PSUM) as pspool,
            tc.tile_pool(name="obuf", bufs=5) as opool,
        ):
            vball = bigp.tile([128, PADC + NSLAB * D], bf16, name="vball")
            vbuf = vball[:, PADC : PADC + NSLAB * D]
            cst_hv = vball[:, 0:PADC].bitcast(f32)

            b0r = lambda s: (cst_hv[:, 2 * s : 2 * s + 1],
                             cst_hv[:, 2 * s + 1 : 2 * s + 2])

            wbuf = bigp.tile([128, NSLAB * WWIN], bf16, name="wbuf")

            # Descriptor generation (SWDGE) costs ~1us per DMA instruction,
            # serialized per queue.  Head DMA (params + V slabs 0-1) on the
            # sync ring; the rest of V is spread over the scalar + gpsimd
            # queues so three SWDGE chains run concurrently.
            nc.sync.dma_start(vball[:, 0 : PADC + D],
                              vp_d[:, 0 : PADC + D])
            nc.sync.dma_start(vball[:, PADC + D : PADC + 3 * D],
                              vp_d[:, PADC + D : PADC + 3 * D])
            nc.scalar.dma_start(vball[:, PADC + 3 * D : PADC + 7 * D],
                                vp_d[:, PADC + 3 * D : PADC + 7 * D])

            # force the erf_derivative ACT table load now (it is inserted
            # right before the first activation in ACT program order; with a
            # no-dependency dummy here it runs while cst is in flight
            # instead of after the cst semaphore wait)
            dummy = constp.tile([128, 1], f32, name="dummy")
            nc.scalar.activation(dummy[:], dummy[:], AF.Derivative_Erf)

            # PE warm-up operands, memset first on gpsimd (ready ~1us before
            # DVE dispatches its first instruction) so the warm-up stream
            # starts at ~6.7us and holds the PE clock ramp without a gap
            wscr = bigp.tile([128, 128], bf16, name="wscr")
            nc.gpsimd.memset(wscr[:], 0.0)
            wscr2 = bigp.tile([128, 128], bf16, name="wscr2")
            nc.gpsimd.memset(wscr2[:], 0.0)

            # iota source row for the W activations, generated on-chip
            # (fp32 is exact for 0..255); needed by W0 at ~9.5us, ready ~8.5
            iota_t = constp.tile([128, WWIN], f32, name="iota_t")
            nc.gpsimd.iota(iota_t[:], [[1, WWIN]], base=0, channel_multiplier=0,
                           allow_small_or_imprecise_dtypes=True)

            # W[p, c] is identically zero for window cols [0,16) and
            # [240,256) of every slab (|c - 64 - p - mu| <= 13.2*sigma is
            # unreachable there), so the per-slab activation covers only the
            # live 224 columns and two strided memsets zero the edge strips
            # once up front (gpsimd, done well before the first matmul)
            nc.gpsimd.memset(wbuf[:].rearrange("p (s c) -> p s c", c=WWIN)[:, :, 0:16], 0.0)
            nc.gpsimd.memset(wbuf[:].rearrange("p (s c) -> p s c", c=WWIN)[:, :, WWIN - 16 : WWIN], 0.0)

            # V tail on gpsimd FIRST (its SWDGE chain starts ~1.5us earlier
            # than if iota/strips ran first), slab order = consumption
            # order; out pairs ride behind V on the sync/gpsimd queues, so
            # V keeps priority
            for eng, lo, hi in ((nc.gpsimd, 7, 12), (nc.gpsimd, 12, 17)):
                eng.dma_start(vbuf[:, lo * D : hi * D],
                              vp_d[:, PADC + lo * D : PADC + hi * D])

            # PE warm-up: many SMALL (128-wide) scratch matmuls on zeros.
            # Fine granularity means the real matmul stream starts at most
            # ~250ns after its inputs are ready (vs 630ns with 512-wide
            # warmups) while PE stays continuously busy through the ~3.5us
            # clock-gate ramp, so real matmuls run at full rate immediately.
            wps = pspool.tile([128, D], f32, tag="ps", name="wps")
            for _ in range(WARMUP):
                nc.tensor.matmul(wps[:, 0:128], wscr[:], wscr2[:],
                                 start=True, stop=True)

            # W slab s in one ACT pass: DErf(r'*u + b0') = (2/sqrt(pi)) *
            # exp(-((u - 64 - p - mu)/sigma)^2 / 2)
            def emit_w(s):
                lo, w = _slab_win(s)
                a, b = max(lo, 16), min(lo + w, WWIN - 16)
                b0, r = b0r(s)
                nc.scalar.activation(
                    wbuf[:, s * WWIN + a : s * WWIN + b],
                    iota_t[:, a:b],
                    AF.Derivative_Erf, bias=b0, scale=r)



            def emit_jtile(t, ps):
                out = ps[:]
                for k in range(2):
                    ls = t + k
                    nc.tensor.matmul(
                        out,
                        wbuf[:, ls * WWIN + (1 - k) * 128 : ls * WWIN + (2 - k) * 128],
                        vbuf[:, ls * D : (ls + 1) * D],
                        start=(k == 0),
                        stop=(k == 1),
                    )

            # Tile does NOT track reads through bitcast APs — without an
            # explicit gate the W chain would race the head DMA and read
            # stale param bytes.  A tracked 1-column ACT copy of the head
            # region blocks ACT until the head semaphore fires.
            gate = constp.tile([128, 1], bf16, name="gate")
            nc.scalar.copy(gate[:], vball[:, 0:1])

            # pipeline: per-slab W -> j-tiles as they unlock -> PSUM->SBUF
            # bf16 copies (DVE tiles 0-12, ACT tiles 13-15 deferred until
            # after the last W slab so the W stream never stalls).
            # Out-DMA granularity: 3 quads + 1 pair + 2 singles = 6 DMA
            # instructions (descriptor generation is ~1us each, serialized
            # per queue — the old 10-instruction scheme spent ~4us of the
            # tail in SWDGE chains).  Quads are 4KB-contiguous per partition
            # in the partition-major out layout.
            emit_w(0)
            ps = ob = None
            deferred = []
            for s in range(1, NSLAB):
                emit_w(s)
                t = s - 1
                psp = pspool.tile([128, D], f32, tag="ps", name="ps")
                if t < 12:
                    if t % 4 == 0:
                        ob = opool.tile([128, 4 * D], bf16, name="ob")
                elif t in (12, 14):
                    ob = opool.tile([128, 2 * D], bf16, name="ob")
                emit_jtile(t, psp)
                if t >= 13:
                    deferred.append((t, psp, ob))
                else:
                    dst = (ob[:, (t % 4) * D : (t % 4 + 1) * D] if t < 12
                           else ob[:, 0:D])
                    nc.vector.tensor_copy(dst, psp[:])
                    if t == 3:
                        nc.sync.dma_start(out_d[:, 0 : 4 * D], ob[:])
                    elif t == 7:
                        nc.gpsimd.dma_start(out_d[:, 4 * D : 8 * D], ob[:])
                    elif t == 11:
                        nc.sync.dma_start(out_d[:, 8 * D : 12 * D], ob[:])
            for t, psp, ob in deferred:
                nc.scalar.activation(
                    ob[:, (t % 2) * D : (t % 2 + 1) * D], psp[:], AF.Copy)
                if t == 13:
                    nc.sync.dma_start(out_d[:, 12 * D : 14 * D], ob[:])
                elif t == 14:
                    nc.gpsimd.dma_start(out_d[:, 14 * D : 15 * D], ob[:, 0:D])
                else:
                    nc.sync.dma_start(out_d[:, 15 * D : 16 * D],
                                      ob[:, D : 2 * D])

    bacc.Bacc.insert_act_table_loads = _single_table_iatl
    try:
        nc.compile()
    finally:
        bacc.Bacc.insert_act_table_loads = _stock_iatl
    return nc


def _get_nc():
    if "nc" not in _cached:
        _cached["nc"] = build_nc()
    return _cached["nc"]


def make_in_maps(V, sigma, mu):
    """Host-side sharding: per-core padded bf16 V rows + scale table."""
    V = np.asarray(V, dtype=np.float32)
    sigma = np.asarray(sigma, dtype=np.float32).reshape(B, N)
    mu = np.asarray(mu, dtype=np.float32).reshape(B, N)
    pidx = (np.arange(VROWS) % 128).astype(np.float32)
    in_maps = []
    for c in range(NCORES):
        b, h = divmod(c, 2)
        jb = h * HALF
        lo, hi = jb - 64, jb + HALF + 64
        slo, shi = max(lo, 0), min(hi, N)
        vp = np.zeros((VROWS, D), ml_dtypes.bfloat16)
        sig = np.ones(VROWS, np.float32)
        muv = np.zeros(VROWS, np.float32)
        vp[slo - lo : shi - lo] = (V[b, slo:shi] * WSCALE).astype(ml_dtypes.bfloat16)
        sig[slo - lo : shi - lo] = sigma[b, slo:shi]
        muv[slo - lo : shi - lo] = mu[b, slo:shi]
        r = (np.float32(1.0) / (sig * np.float32(SQRT2))).astype(np.float32)
        b0 = ((np.float32(-64.0) - pidx - muv) * r).astype(np.float32)
        cst = np.zeros((128, CW), np.float32)
        cst[:, 0 : 2 * NSLAB : 2] = b0.reshape(NSLAB, 128).T
        cst[:, 1 : 2 * NSLAB : 2] = r.reshape(NSLAB, 128).T
        vp2 = np.empty((128, PADC + NSLAB * D), ml_dtypes.bfloat16)
        # f32 params shipped as raw bytes in the bf16 tensor's first columns
        vp2[:, 0:PADC] = np.ascontiguousarray(cst).view(np.uint16).view(
            ml_dtypes.bfloat16)
        vp2[:, PADC:] = (
            vp.reshape(NSLAB, 128, D).transpose(1, 0, 2).reshape(128, NSLAB * D))
        in_maps.append({"Vp": vp2})
    return in_maps


def gather(results):
    out = np.empty((B, N, D), np.float32)
    for c in range(NCORES):
        b, h = divmod(c, 2)
        arr = np.asarray(results[c]["out"]).astype(np.float32)
        out[b, h * HALF : (h + 1) * HALF] = (
            arr.reshape(128, JT, D).transpose(1, 0, 2).reshape(HALF, D))
    return out


def kernel(V, sigma, mu):
    nc = _get_nc()
    in_maps = make_in_maps(V, sigma, mu)
    res = run_bass_kernel_spmd(nc, in_maps, core_ids=list(range(NCORES)))
    return gather(res.results)
